# revision 1
# baseline (speedup 1.0000x reference)
"""Multi-head attention (B=2, S=2048, D=1024, H=16, Hd=64) on 8 trn2 cores.

Sharding: core c handles batch b = c // 4 and head group g = c % 4 (heads
4g..4g+3).  Each core computes a partial output  O_g @ Wo[256g:256g+256, :]
for its batch; the host sums the 4 partials per batch and adds the
constant vector  bv @ Wo + bo  (exact: softmax weights sum to 1, so the
V-bias contributes bv @ Wo; bk cancels inside softmax rows).

Device program (identical on all cores, SPMD over different data):
  - inputs (per core): xT [1024,2048] bf16 (x[b].T), wq/wk/wv [1024,256]
    bf16 (head-group column slices; wq and bq pre-scaled by 1/8),
    wo [4,64,1024] bf16 (head-major row slice of Wo), bq [128,2] f32.
  - Q^T,K^T = W^T @ x^T  (contraction over d_in on partitions),
    V = x @ Wv (natural orientation), with a constant ones column
    appended per head for softmax row sums.
  - per (head, 512-wide query chunk): S^T[k,q] via c=64 matmuls,
    exp on ScalarE (PSUM->SBUF bf16, no max subtraction: |scores|<~3),
    O^T/sums accumulated over 16 k-blocks via lhsT=[V_h|1], normalize
    via reciprocal + PE broadcast matmul.
  - final: out[s,:] = sum_h O_h @ Wo_h  (c=64 accumulating matmuls).
"""

import numpy as np
import ml_dtypes

import concourse.bass as bass
import concourse.mybir as mybir
import concourse.tile as tile
from concourse import bacc
from concourse.bass_utils import run_bass_kernel_spmd

BF16 = mybir.dt.bfloat16
F32 = mybir.dt.float32
NPBF16 = ml_dtypes.bfloat16

S = 2048
D = 1024
HG = 4        # heads per core
HD = 64       # head dim
DG = HG * HD  # 256 projection columns per core

# Set by test.py to capture a profile; results stashed on LAST_RESULTS.
TRACE = False
TRACE_KWARGS = {}
LAST_RESULTS = None

_PROGRAM = None


def _emit(tc, xT, wq, wk, wv, wo, bq, out):
    nc = tc.nc
    from contextlib import ExitStack

    with ExitStack() as ctx:
        const = ctx.enter_context(tc.tile_pool(name="const", bufs=1))
        sb = ctx.enter_context(tc.tile_pool(name="sb", bufs=1))
        pts = ctx.enter_context(tc.tile_pool(name="pt", bufs=2))
        outp = ctx.enter_context(tc.tile_pool(name="outp", bufs=3))
        rbp = ctx.enter_context(tc.tile_pool(name="rb", bufs=2))
        psp = ctx.enter_context(
            tc.tile_pool(name="ps", bufs=1, space=bass.MemorySpace.PSUM)
        )
        pop = ctx.enter_context(
            tc.tile_pool(name="po", bufs=1, space=bass.MemorySpace.PSUM)
        )
        pbp = ctx.enter_context(
            tc.tile_pool(name="pb", bufs=1, space=bass.MemorySpace.PSUM)
        )

        # ---- persistent SBUF tensors ----
        x_sb = sb.tile([128, 8, S], BF16)          # x^T, d_in blocks on partitions
        wq_sb = const.tile([128, 8, DG], BF16)
        wk_sb = const.tile([128, 8, DG], BF16)
        wv_sb = const.tile([128, 8, DG], BF16)
        wo_sb = const.tile([64, HG, D], BF16)      # head-major Wo rows
        bq_sb = const.tile([128, 2], F32)
        ones_sb = const.tile([128, 64], F32)       # for sums broadcast lhsT
        qt_sb = sb.tile([128, 2, S], BF16)         # Q^T (d_out on partitions)
        kt_sb = sb.tile([128, 2, S], BF16)         # K^T
        v_sb = sb.tile([128, 16, HG, HD + 1], BF16)  # V by s-block, ones col last
        o_sb = sb.tile([64, HG, S], BF16)          # O^T per head (partitions 0-63)

        # ---- input DMAs ----
        nc.sync.dma_start(out=wq_sb[:], in_=wq.rearrange("(j p) m -> p j m", p=128))
        nc.sync.dma_start(out=wk_sb[:], in_=wk.rearrange("(j p) m -> p j m", p=128))
        nc.sync.dma_start(out=wv_sb[:], in_=wv.rearrange("(j p) m -> p j m", p=128))
        nc.sync.dma_start(out=wo_sb[:], in_=wo.rearrange("h p m -> p h m"))
        nc.sync.dma_start(out=bq_sb[:], in_=bq)
        nc.vector.memset(ones_sb[:], 1.0)
        nc.vector.memset(v_sb[:, :, :, HD : HD + 1], 1.0)
        xT_r = xT.rearrange("(j p) s -> p j s", p=128)
        for c in range(4):
            cs = slice(512 * c, 512 * (c + 1))
            nc.sync.dma_start(out=x_sb[:, :, cs], in_=xT_r[:, :, cs])

        # PSUM: 6 rotating banks + PV bank + broadcast bank = 8
        ps = psp.tile([128, 6, 512], F32)
        slot = [0]

        def nslot():
            s0 = slot[0]
            slot[0] = (s0 + 1) % 6
            return s0

        # ---- phase 1: projections ----
        for c in range(4):
            cs = slice(512 * c, 512 * (c + 1))
            for j2 in range(2):
                ms = slice(128 * j2, 128 * (j2 + 1))
                acc = ps[:, nslot(), :]
                for j in range(8):
                    nc.tensor.matmul(
                        acc, wq_sb[:, j, ms], x_sb[:, j, cs],
                        start=(j == 0), stop=(j == 7),
                    )
                nc.vector.tensor_scalar_add(
                    qt_sb[:, j2, cs], acc, bq_sb[:, j2 : j2 + 1]
                )
                acc = ps[:, nslot(), :]
                for j in range(8):
                    nc.tensor.matmul(
                        acc, wk_sb[:, j, ms], x_sb[:, j, cs],
                        start=(j == 0), stop=(j == 7),
                    )
                nc.vector.tensor_copy(kt_sb[:, j2, cs], acc)
            for sblk in range(4 * c, 4 * c + 4):
                ss = slice(128 * sblk, 128 * (sblk + 1))
                acc = ps[:, nslot(), 0:DG]
                for j in range(8):
                    nc.tensor.matmul(
                        acc, x_sb[:, j, ss], wv_sb[:, j, :],
                        start=(j == 0), stop=(j == 7),
                    )
                nc.vector.tensor_copy(
                    v_sb[:, sblk, :, 0:HD],
                    acc.rearrange("p (h d) -> p h d", h=HG),
                )

        # ---- phase 2+3: attention per query chunk, then final projection ----
        slot[0] = 0
        for c in range(4):
            cs = slice(512 * c, 512 * (c + 1))
            for h in range(HG):
                pp = slice(64 * (h % 2), 64 * (h % 2) + 64)
                j2 = h // 2
                pt = pts.tile([128, 16, 512], BF16)
                po = pop.tile([128, 512], F32)
                # scores^T + exp, two k-blocks per activation
                for kb2 in range(8):
                    s0 = nslot()
                    for ki in range(2):
                        kblk = 2 * kb2 + ki
                        ks = slice(128 * kblk, 128 * (kblk + 1))
                        nc.tensor.matmul(
                            ps[:, s0 + ki, :],
                            kt_sb[pp, j2, ks],
                            qt_sb[pp, j2, cs],
                            start=True, stop=True,
                        )
                    nslot()
                    nc.scalar.activation(
                        out=pt[:, 2 * kb2 : 2 * kb2 + 2, :],
                        in_=ps[:, s0 : s0 + 2, :],
                        func=mybir.ActivationFunctionType.Exp,
                    )
                # P^T.T @ [V_h | 1] -> O^T rows 0-63, sums row 64
                for kblk in range(16):
                    nc.tensor.matmul(
                        po[0 : HD + 1, :],
                        v_sb[:, kblk, h, :],
                        pt[:, kblk, :],
                        start=(kblk == 0), stop=(kblk == 15),
                    )
                # normalize: o = o * (1/sums) broadcast over partitions
                rb = rbp.tile([128, 512], F32)
                nc.vector.reciprocal(rb[HD : HD + 1, :], po[HD : HD + 1, :])
                pb = pbp.tile([64, 512], F32)
                nc.tensor.matmul(
                    pb[:],
                    ones_sb[HD : HD + 1, :],
                    rb[HD : HD + 1, :],
                    start=True, stop=True,
                )
                nc.vector.tensor_copy(rb[0:64, :], pb[:])
                nc.vector.tensor_mul(o_sb[:, h, cs], po[0:HD, :], rb[0:64, :])
            # final projection for the 4 s-blocks of this chunk
            for sblk in range(4 * c, 4 * c + 4):
                ss = slice(128 * sblk, 128 * (sblk + 1))
                ot = outp.tile([128, D], F32)
                for half in range(2):
                    hs = slice(512 * half, 512 * (half + 1))
                    acc = ps[:, nslot(), :]
                    for h in range(HG):
                        nc.tensor.matmul(
                            acc, o_sb[:, h, ss], wo_sb[:, h, hs],
                            start=(h == 0), stop=(h == HG - 1),
                        )
                    nc.vector.tensor_copy(ot[:, hs], acc)
                nc.sync.dma_start(out=out[ss, :], in_=ot[:])


def _build_program():
    nc = bacc.Bacc(
        "TRN2", target_bir_lowering=False, debug=False, num_devices=8
    )
    xT = nc.dram_tensor("xT", [D, S], BF16, kind="ExternalInput").ap()
    wq = nc.dram_tensor("wq", [D, DG], BF16, kind="ExternalInput").ap()
    wk = nc.dram_tensor("wk", [D, DG], BF16, kind="ExternalInput").ap()
    wv = nc.dram_tensor("wv", [D, DG], BF16, kind="ExternalInput").ap()
    wo = nc.dram_tensor("wo", [HG, HD, D], BF16, kind="ExternalInput").ap()
    bq = nc.dram_tensor("bq", [128, 2], F32, kind="ExternalInput").ap()
    out = nc.dram_tensor("out", [S, D], F32, kind="ExternalOutput").ap()
    with tile.TileContext(nc) as tc:
        _emit(tc, xT, wq, wk, wv, wo, bq, out)
    nc.compile()
    return nc


def _get_program():
    global _PROGRAM
    if _PROGRAM is None:
        _PROGRAM = _build_program()
    return _PROGRAM


def kernel(x, Wq, bq, Wk, bk, Wv, bv, Wo, bo):
    global LAST_RESULTS
    x = np.asarray(x, dtype=np.float32)
    Wq = np.asarray(Wq, dtype=np.float32)
    bq = np.asarray(bq, dtype=np.float32)
    Wk = np.asarray(Wk, dtype=np.float32)
    Wv = np.asarray(Wv, dtype=np.float32)
    bv = np.asarray(bv, dtype=np.float32)
    Wo = np.asarray(Wo, dtype=np.float32)
    bo = np.asarray(bo, dtype=np.float32)

    scale = 1.0 / np.sqrt(np.float32(HD)).astype(np.float32)
    nc = _get_program()

    xT_b = [np.ascontiguousarray(x[b].T).astype(NPBF16) for b in range(2)]
    in_maps = []
    for c in range(8):
        b, g = c // 4, c % 4
        hs = slice(DG * g, DG * (g + 1))
        in_maps.append(
            {
                "xT": xT_b[b],
                "wq": (Wq[:, hs] * scale).astype(NPBF16),
                "wk": np.ascontiguousarray(Wk[:, hs]).astype(NPBF16),
                "wv": np.ascontiguousarray(Wv[:, hs]).astype(NPBF16),
                "wo": np.ascontiguousarray(
                    Wo[hs, :].reshape(HG, HD, D)
                ).astype(NPBF16),
                "bq": np.ascontiguousarray(
                    (bq[hs] * scale).reshape(2, 128).T
                ).astype(np.float32),
            }
        )

    res = run_bass_kernel_spmd(
        nc, in_maps, list(range(8)), trace=TRACE, **TRACE_KWARGS
    )
    LAST_RESULTS = res
    parts = [r["out"] for r in res.results]
    const = (bv @ Wo + bo).astype(np.float32)
    y = np.empty((2, S, D), dtype=np.float32)
    for b in range(2):
        y[b] = parts[4 * b] + parts[4 * b + 1] + parts[4 * b + 2] + parts[4 * b + 3]
        y[b] += const
    return y


# revision 6
# speedup vs baseline: 1.0832x; 1.0832x over previous
"""Multi-head attention (B=2, S=2048, D=1024, H=16, Hd=64) on 8 trn2 cores.

Sharding: core c handles batch b = c // 4 and head group g = c % 4 (heads
4g..4g+3).  Each core computes a partial output  O_g @ Wo[256g:256g+256, :]
for its batch; the host sums the 4 partials per batch and adds the
constant vector  bv @ Wo + bo  (exact: softmax weights sum to 1, so the
V-bias contributes bv @ Wo; bk cancels inside softmax rows).

Device program (identical on all cores, SPMD over different data):
  - inputs (per core): xT [1024,2048] bf16 (x[b].T), wq/wk/wv [1024,256]
    bf16 (head-group column slices; wq and bq pre-scaled by 1/8),
    wo [4,64,1024] bf16 (head-major row slice of Wo), bq [128,2] f32.
  - Q^T,K^T = W^T @ x^T  (contraction over d_in on partitions),
    V = x @ Wv (natural orientation), with a constant ones column
    appended per head for softmax row sums.
  - per (head, 512-wide query chunk): S^T[k,q] via c=64 matmuls,
    exp on ScalarE (PSUM->SBUF bf16, no max subtraction: |scores|<~3),
    O^T/sums accumulated over 16 k-blocks via lhsT=[V_h|1], normalize
    via reciprocal + PE broadcast matmul.
  - final: out[s,:] = sum_h O_h @ Wo_h  (c=64 accumulating matmuls).
"""

import numpy as np
import ml_dtypes

import concourse.bass as bass
import concourse.mybir as mybir
import concourse.tile as tile
from concourse import bacc
from concourse.bass_utils import run_bass_kernel_spmd

BF16 = mybir.dt.bfloat16
F32 = mybir.dt.float32
NPBF16 = ml_dtypes.bfloat16

S = 2048
D = 1024
HG = 4        # heads per core
HD = 64       # head dim
DG = HG * HD  # 256 projection columns per core

USE_FAST_RECIP = False

# Set by test.py to capture a profile; results stashed on LAST_RESULTS.
TRACE = False
TRACE_KWARGS = {}
LAST_RESULTS = None

_PROGRAM = None


def _emit(tc, xT, wq, wk, wv, wo, bq, out):
    nc = tc.nc
    from contextlib import ExitStack

    with ExitStack() as ctx:
        const = ctx.enter_context(tc.tile_pool(name="const", bufs=1))
        sb = ctx.enter_context(tc.tile_pool(name="sb", bufs=1))
        pts = ctx.enter_context(tc.tile_pool(name="pt", bufs=2))
        outp = ctx.enter_context(tc.tile_pool(name="outp", bufs=3))
        rbp = ctx.enter_context(tc.tile_pool(name="rb", bufs=2))
        psp = ctx.enter_context(
            tc.tile_pool(name="ps", bufs=1, space=bass.MemorySpace.PSUM)
        )
        pop = ctx.enter_context(
            tc.tile_pool(name="po", bufs=1, space=bass.MemorySpace.PSUM)
        )
        pbp = ctx.enter_context(
            tc.tile_pool(name="pb", bufs=2, space=bass.MemorySpace.PSUM)
        )

        # ---- persistent SBUF tensors ----
        x_sb = sb.tile([128, 8, S], BF16)          # x^T, d_in blocks on partitions
        wq_sb = const.tile([128, 8, DG], BF16)
        wk_sb = const.tile([128, 8, DG], BF16)
        wv_sb = const.tile([128, 8, DG], BF16)
        wo_sb = const.tile([64, HG, D], BF16)      # head-major Wo rows
        bq_sb = const.tile([128, 2], F32)
        ones_sb = const.tile([128, 64], F32)       # for sums broadcast lhsT
        qt_sb = sb.tile([128, 2, S], BF16)         # Q^T (d_out on partitions)
        kt_sb = sb.tile([128, 2, S], BF16)         # K^T
        v_sb = sb.tile([128, 16, HG, HD + 1], BF16)  # V by s-block, ones col last
        o_sb = sb.tile([64, HG, S], BF16)          # O^T per head (partitions 0-63)

        # ---- input DMAs ----
        nc.sync.dma_start(out=wq_sb[:], in_=wq.rearrange("(j p) m -> p j m", p=128))
        nc.sync.dma_start(out=wk_sb[:], in_=wk.rearrange("(j p) m -> p j m", p=128))
        nc.sync.dma_start(out=wv_sb[:], in_=wv.rearrange("(j p) m -> p j m", p=128))
        nc.sync.dma_start(out=wo_sb[:], in_=wo.rearrange("h p m -> p h m"))
        nc.sync.dma_start(out=bq_sb[:], in_=bq)
        nc.vector.memset(ones_sb[:], 1.0)
        nc.vector.memset(v_sb[:, :, :, HD : HD + 1], 1.0)
        xT_r = xT.rearrange("(j p) s -> p j s", p=128)
        for c in range(4):
            cs = slice(512 * c, 512 * (c + 1))
            nc.sync.dma_start(out=x_sb[:, :, cs], in_=xT_r[:, :, cs])

        # PSUM: 4 rotating score banks + 2 PV banks + 2 broadcast banks = 8
        NPS = 4
        ps = psp.tile([128, NPS, 512], F32)
        slot = [0]

        def nslot():
            s0 = slot[0]
            slot[0] = (s0 + 1) % NPS
            return s0

        # ---- phase 1: projections ----
        for c in range(4):
            cs = slice(512 * c, 512 * (c + 1))
            for j2 in range(2):
                ms = slice(128 * j2, 128 * (j2 + 1))
                acc = ps[:, nslot(), :]
                for j in range(8):
                    nc.tensor.matmul(
                        acc, wq_sb[:, j, ms], x_sb[:, j, cs],
                        start=(j == 0), stop=(j == 7),
                    )
                nc.vector.tensor_scalar_add(
                    qt_sb[:, j2, cs], acc, bq_sb[:, j2 : j2 + 1]
                )
                acc = ps[:, nslot(), :]
                for j in range(8):
                    nc.tensor.matmul(
                        acc, wk_sb[:, j, ms], x_sb[:, j, cs],
                        start=(j == 0), stop=(j == 7),
                    )
                nc.vector.tensor_copy(kt_sb[:, j2, cs], acc)
            for sblk in range(4 * c, 4 * c + 4):
                ss = slice(128 * sblk, 128 * (sblk + 1))
                acc = ps[:, nslot(), 0:DG]
                for j in range(8):
                    nc.tensor.matmul(
                        acc, x_sb[:, j, ss], wv_sb[:, j, :],
                        start=(j == 0), stop=(j == 7),
                    )
                nc.vector.tensor_copy(
                    v_sb[:, sblk, :, 0:HD],
                    acc.rearrange("p (h d) -> p h d", h=HG),
                )

        # ---- phase 2+3: attention per query chunk, then final projection ----
        # Heads processed in pairs (2*j2, 2*j2+1): the two c=64 score
        # matmuls land on PE row groups 0-63 / 64-127 (tile_position
        # auto-derived from base_partition) and run concurrently.
        slot[0] = 0
        for c in range(4):
            cs = slice(512 * c, 512 * (c + 1))
            for j2 in range(2):
                pt = pts.tile([128, 16, 2, 512], BF16)
                po = pop.tile([128, 2, 512], F32)

                def pv(kblk, j2=j2, po=po, pt=pt):
                    for hh in range(2):
                        nc.tensor.matmul(
                            po[0 : HD + 1, hh, :],
                            v_sb[:, kblk, 2 * j2 + hh, :],
                            pt[:, kblk, hh, :],
                            start=(kblk == 0), stop=(kblk == 15),
                        )

                for kblk in range(16):
                    ks = slice(128 * kblk, 128 * (kblk + 1))
                    s0 = 2 * (kblk % 2)
                    for hh in range(2):
                        pp = slice(64 * hh, 64 * hh + 64)
                        nc.tensor.matmul(
                            ps[:, s0 + hh, :],
                            kt_sb[pp, j2, ks],
                            qt_sb[pp, j2, cs],
                            start=True, stop=True,
                        )
                    nc.scalar.activation(
                        out=pt[:, kblk, :, :],
                        in_=ps[:, s0 : s0 + 2, :],
                        func=mybir.ActivationFunctionType.Exp,
                    )
                    if kblk >= 1:
                        pv(kblk - 1)
                pv(15)
                # normalize: o_h = o_h * (1/sums_h) broadcast over partitions
                for hh in range(2):
                    h = 2 * j2 + hh
                    rb = rbp.tile([128, 512], F32)
                    if USE_FAST_RECIP:
                        nc.vector.reciprocal_approx_fast(
                            rb[HD : HD + 1, :], po[HD : HD + 1, hh, :]
                        )
                    else:
                        nc.vector.reciprocal(
                            rb[HD : HD + 1, :], po[HD : HD + 1, hh, :]
                        )
                    pb = pbp.tile([64, 512], F32)
                    nc.tensor.matmul(
                        pb[:],
                        ones_sb[HD : HD + 1, :],
                        rb[HD : HD + 1, :],
                        start=True, stop=True,
                    )
                    nc.vector.tensor_copy(rb[0:64, :], pb[:])
                    nc.vector.tensor_mul(
                        o_sb[:, h, cs], po[0:HD, hh, :], rb[0:64, :]
                    )
            # final projection for the 4 s-blocks of this chunk
            for sblk in range(4 * c, 4 * c + 4):
                ss = slice(128 * sblk, 128 * (sblk + 1))
                ot = outp.tile([128, D], F32)
                for half in range(2):
                    hs = slice(512 * half, 512 * (half + 1))
                    acc = ps[:, nslot(), :]
                    for h in range(HG):
                        nc.tensor.matmul(
                            acc, o_sb[:, h, ss], wo_sb[:, h, hs],
                            start=(h == 0), stop=(h == HG - 1),
                        )
                    nc.vector.tensor_copy(ot[:, hs], acc)
                nc.sync.dma_start(out=out[ss, :], in_=ot[:])


def _build_program():
    nc = bacc.Bacc(
        "TRN2", target_bir_lowering=False, debug=False, num_devices=8
    )
    xT = nc.dram_tensor("xT", [D, S], BF16, kind="ExternalInput").ap()
    wq = nc.dram_tensor("wq", [D, DG], BF16, kind="ExternalInput").ap()
    wk = nc.dram_tensor("wk", [D, DG], BF16, kind="ExternalInput").ap()
    wv = nc.dram_tensor("wv", [D, DG], BF16, kind="ExternalInput").ap()
    wo = nc.dram_tensor("wo", [HG, HD, D], BF16, kind="ExternalInput").ap()
    bq = nc.dram_tensor("bq", [128, 2], F32, kind="ExternalInput").ap()
    out = nc.dram_tensor("out", [S, D], F32, kind="ExternalOutput").ap()
    with tile.TileContext(nc) as tc:
        _emit(tc, xT, wq, wk, wv, wo, bq, out)
    nc.compile()
    return nc


def _get_program():
    global _PROGRAM
    if _PROGRAM is None:
        _PROGRAM = _build_program()
    return _PROGRAM


def kernel(x, Wq, bq, Wk, bk, Wv, bv, Wo, bo):
    global LAST_RESULTS
    x = np.asarray(x, dtype=np.float32)
    Wq = np.asarray(Wq, dtype=np.float32)
    bq = np.asarray(bq, dtype=np.float32)
    Wk = np.asarray(Wk, dtype=np.float32)
    Wv = np.asarray(Wv, dtype=np.float32)
    bv = np.asarray(bv, dtype=np.float32)
    Wo = np.asarray(Wo, dtype=np.float32)
    bo = np.asarray(bo, dtype=np.float32)

    scale = 1.0 / np.sqrt(np.float32(HD)).astype(np.float32)
    nc = _get_program()

    xT_b = [np.ascontiguousarray(x[b].T).astype(NPBF16) for b in range(2)]
    in_maps = []
    for c in range(8):
        b, g = c // 4, c % 4
        hs = slice(DG * g, DG * (g + 1))
        in_maps.append(
            {
                "xT": xT_b[b],
                "wq": (Wq[:, hs] * scale).astype(NPBF16),
                "wk": np.ascontiguousarray(Wk[:, hs]).astype(NPBF16),
                "wv": np.ascontiguousarray(Wv[:, hs]).astype(NPBF16),
                "wo": np.ascontiguousarray(
                    Wo[hs, :].reshape(HG, HD, D)
                ).astype(NPBF16),
                "bq": np.ascontiguousarray(
                    (bq[hs] * scale).reshape(2, 128).T
                ).astype(np.float32),
            }
        )

    res = run_bass_kernel_spmd(
        nc, in_maps, list(range(8)), trace=TRACE, **TRACE_KWARGS
    )
    LAST_RESULTS = res
    parts = [r["out"] for r in res.results]
    const = (bv @ Wo + bo).astype(np.float32)
    y = np.empty((2, S, D), dtype=np.float32)
    for b in range(2):
        y[b] = parts[4 * b] + parts[4 * b + 1] + parts[4 * b + 2] + parts[4 * b + 3]
        y[b] += const
    return y


# revision 7
# speedup vs baseline: 1.1908x; 1.0993x over previous
"""Multi-head attention (B=2, S=2048, D=1024, H=16, Hd=64) on 8 trn2 cores.

Sharding: core c handles batch b = c // 4 and head group g = c % 4 (heads
4g..4g+3).  Each core computes a partial output  O_g @ Wo[256g:256g+256, :]
for its batch; the host sums the 4 partials per batch and adds the
constant vector  bv @ Wo + bo  (exact: softmax weights sum to 1, so the
V-bias contributes bv @ Wo; bk cancels inside softmax rows).

Device program (identical on all cores, SPMD over different data):
  - inputs (per core): xT [1024,2048] bf16 (x[b].T), wq/wk/wv [1024,256]
    bf16 (head-group column slices; wq and bq pre-scaled by 1/8),
    wo [4,64,1024] bf16 (head-major row slice of Wo), bq [128,2] f32.
  - Q^T,K^T = W^T @ x^T  (contraction over d_in on partitions),
    V = x @ Wv (natural orientation), with a constant ones column
    appended per head for softmax row sums.
  - per (head pair 2*j2..2*j2+1, 512-wide query chunk): S^T[k,q] via
    c=64 matmuls packed on PE row groups 0-63/64-127, exp on ScalarE
    (PSUM->SBUF bf16, no max subtraction: |scores| < ~3), O^T/sums
    accumulated over 16 k-blocks via lhsT=[V_h|1].
  - softmax denominators: sums row spread over 128 partitions via a
    small SBUF->SBUF DMA, reciprocal there (cheap), DMA back, then a
    c=1 PE matmul broadcasts 1/sum over partitions; one DVE multiply
    normalizes.  The broadcast matmul and the final projection are
    emitted one group late so the in-order PE queue never stalls on
    the normalization tail.
  - final: out[s,:] = sum_h O_h @ Wo_h  (c=64 accumulating matmuls).
"""

import numpy as np
import ml_dtypes

import concourse.bass as bass
import concourse.mybir as mybir
import concourse.tile as tile
from concourse import bacc
from concourse.bass_utils import run_bass_kernel_spmd

BF16 = mybir.dt.bfloat16
F32 = mybir.dt.float32
NPBF16 = ml_dtypes.bfloat16

S = 2048
D = 1024
HG = 4        # heads per core
HD = 64       # head dim
DG = HG * HD  # 256 projection columns per core

# Set by test.py to capture a profile; results stashed on LAST_RESULTS.
TRACE = False
TRACE_KWARGS = {}
LAST_RESULTS = None

_PROGRAM = None


def _emit(tc, xT, wq, wk, wv, wo, bq, out):
    nc = tc.nc
    from contextlib import ExitStack

    with ExitStack() as ctx:
        const = ctx.enter_context(tc.tile_pool(name="const", bufs=1))
        sb = ctx.enter_context(tc.tile_pool(name="sb", bufs=1))
        pts = ctx.enter_context(tc.tile_pool(name="pt", bufs=2))
        outp = ctx.enter_context(tc.tile_pool(name="outp", bufs=3))
        obp = ctx.enter_context(tc.tile_pool(name="ob", bufs=2))
        rsp = ctx.enter_context(tc.tile_pool(name="rs", bufs=2))
        rrp = ctx.enter_context(tc.tile_pool(name="rr", bufs=2))
        psp = ctx.enter_context(
            tc.tile_pool(name="ps", bufs=1, space=bass.MemorySpace.PSUM)
        )
        pop = ctx.enter_context(
            tc.tile_pool(name="po", bufs=1, space=bass.MemorySpace.PSUM)
        )

        # ---- persistent SBUF tensors ----
        x_sb = sb.tile([128, 8, S], BF16)          # x^T, d_in blocks on partitions
        wq_sb = const.tile([128, 8, DG], BF16)
        wk_sb = const.tile([128, 8, DG], BF16)
        wv_sb = const.tile([128, 8, DG], BF16)
        wo_sb = const.tile([64, HG, D], BF16)      # head-major Wo rows
        bq_sb = const.tile([128, 2], F32)
        ones_sb = const.tile([128, 64], F32)       # for 1/sum broadcast lhsT
        scr_sb = const.tile([1, 1], F32)
        qt_sb = sb.tile([128, 2, S], BF16)         # Q^T (d_out on partitions)
        kt_sb = sb.tile([128, 2, S], BF16)         # K^T
        v_sb = sb.tile([128, 16, HG, HD + 1], BF16)  # V by s-block, ones col last
        o_sb = sb.tile([64, HG, S], BF16)          # O^T per head (partitions 0-63)

        # ---- input DMAs (ordered so phase-1 compute can start early) ----
        nc.vector.memset(ones_sb[:], 1.0)
        nc.vector.memset(v_sb[:, :, :, HD : HD + 1], 1.0)
        # preload the exp table set while DMAs run
        nc.scalar.activation(
            out=scr_sb[:], in_=ones_sb[0:1, 0:1],
            func=mybir.ActivationFunctionType.Exp,
        )
        xT_r = xT.rearrange("(j p) s -> p j s", p=128)
        nc.sync.dma_start(out=wq_sb[:], in_=wq.rearrange("(j p) m -> p j m", p=128))
        nc.sync.dma_start(out=wk_sb[:], in_=wk.rearrange("(j p) m -> p j m", p=128))
        nc.sync.dma_start(out=bq_sb[:], in_=bq)
        nc.sync.dma_start(out=x_sb[:, :, 0:512], in_=xT_r[:, :, 0:512])
        nc.sync.dma_start(out=wv_sb[:], in_=wv.rearrange("(j p) m -> p j m", p=128))
        nc.sync.dma_start(out=wo_sb[:], in_=wo.rearrange("h p m -> p h m"))
        for c in range(1, 4):
            cs = slice(512 * c, 512 * (c + 1))
            nc.sync.dma_start(out=x_sb[:, :, cs], in_=xT_r[:, :, cs])

        # PSUM: 6 rotating banks (scores + 1/sum broadcast + final) + 2 PV banks
        NPS = 6
        ps = psp.tile([128, NPS, 512], F32)
        slot = [0]

        def nslot():
            s0 = slot[0]
            slot[0] = (s0 + 1) % NPS
            return s0

        # ---- phase 1: projections ----
        for c in range(4):
            cs = slice(512 * c, 512 * (c + 1))
            for j2 in range(2):
                ms = slice(128 * j2, 128 * (j2 + 1))
                acc = ps[:, nslot(), :]
                for j in range(8):
                    nc.tensor.matmul(
                        acc, wq_sb[:, j, ms], x_sb[:, j, cs],
                        start=(j == 0), stop=(j == 7),
                    )
                nc.vector.tensor_scalar_add(
                    qt_sb[:, j2, cs], acc, bq_sb[:, j2 : j2 + 1]
                )
                acc = ps[:, nslot(), :]
                for j in range(8):
                    nc.tensor.matmul(
                        acc, wk_sb[:, j, ms], x_sb[:, j, cs],
                        start=(j == 0), stop=(j == 7),
                    )
                nc.vector.tensor_copy(kt_sb[:, j2, cs], acc)
            for sblk in range(4 * c, 4 * c + 4):
                ss = slice(128 * sblk, 128 * (sblk + 1))
                acc = ps[:, nslot(), 0:DG]
                for j in range(8):
                    nc.tensor.matmul(
                        acc, x_sb[:, j, ss], wv_sb[:, j, :],
                        start=(j == 0), stop=(j == 7),
                    )
                nc.vector.tensor_copy(
                    v_sb[:, sblk, :, 0:HD],
                    acc.rearrange("p (h d) -> p h d", h=HG),
                )

        # ---- phase 2+3: attention, with norm-tail and final projection
        # deferred by one group to keep the PE queue unblocked ----
        slot[0] = 0

        def npair():
            s0 = slot[0]
            assert s0 % 2 == 0
            slot[0] = (s0 + 2) % NPS
            return s0

        def emit_final(c):
            for sblk in range(4 * c, 4 * c + 4):
                ss = slice(128 * sblk, 128 * (sblk + 1))
                ot = outp.tile([128, D], F32)
                s0 = npair()
                for half in range(2):
                    hs = slice(512 * half, 512 * (half + 1))
                    acc = ps[:, s0 + half, :]
                    for h in range(HG):
                        nc.tensor.matmul(
                            acc, o_sb[:, h, ss], wo_sb[:, h, hs],
                            start=(h == 0), stop=(h == HG - 1),
                        )
                    nc.vector.tensor_copy(ot[:, hs], acc)
                nc.sync.dma_start(out=out[ss, :], in_=ot[:])

        pending = []  # deferred norm-tail emitters
        for c in range(4):
            cs = slice(512 * c, 512 * (c + 1))
            for j2 in range(2):
                pt = pts.tile([128, 16, 2, 512], BF16)
                po = pop.tile([128, 2, 512], F32)

                def pv(kblk, j2=j2, po=po, pt=pt):
                    for hh in range(2):
                        nc.tensor.matmul(
                            po[0 : HD + 1, hh, :],
                            v_sb[:, kblk, 2 * j2 + hh, :],
                            pt[:, kblk, hh, :],
                            start=(kblk == 0), stop=(kblk == 15),
                        )

                for kblk in range(16):
                    ks = slice(128 * kblk, 128 * (kblk + 1))
                    s0 = npair()
                    for hh in range(2):
                        pp = slice(64 * hh, 64 * hh + 64)
                        nc.tensor.matmul(
                            ps[:, s0 + hh, :],
                            kt_sb[pp, j2, ks],
                            qt_sb[pp, j2, cs],
                            start=True, stop=True,
                        )
                    nc.scalar.activation(
                        out=pt[:, kblk, :, :],
                        in_=ps[:, s0 : s0 + 2, :],
                        func=mybir.ActivationFunctionType.Exp,
                    )
                    if kblk >= 1:
                        pv(kblk - 1)
                pv(15)

                # evacuate PV banks; spread sums over partitions via DMA,
                # reciprocal there, DMA back for the broadcast matmul
                ob = obp.tile([HD + 1, 2, 512], F32)
                nc.vector.tensor_copy(ob[:], po[0 : HD + 1, :, :])
                rs = rsp.tile([128, 2, 4], F32)
                rr = rrp.tile([128, 2, 512], F32)
                for hh in range(2):
                    nc.sync.dma_start(
                        out=rs[:, hh, :], in_=ob[HD : HD + 1, hh, :]
                    )
                nc.vector.reciprocal(rs[:], rs[:])
                for hh in range(2):
                    nc.sync.dma_start(
                        out=rr[HD : HD + 1, hh, :], in_=rs[:, hh, :]
                    )

                def tail(c=c, cs=cs, j2=j2, ob=ob, rr=rr):
                    for hh in range(2):
                        s0h = nslot()
                        nc.tensor.matmul(
                            ps[0:64, s0h, :],
                            ones_sb[HD : HD + 1, :],
                            rr[HD : HD + 1, hh, :],
                            start=True, stop=True,
                        )
                        nc.vector.tensor_mul(
                            o_sb[:, 2 * j2 + hh, cs],
                            ob[0:HD, hh, :],
                            ps[0:64, s0h, :],
                        )

                pending.append((tail, c if j2 == 1 else None))
                if len(pending) >= 2:
                    t, fc = pending.pop(0)
                    t()
                    if fc is not None:
                        emit_final(fc)
        for t, fc in pending:
            t()
            if fc is not None:
                emit_final(fc)


def _build_program():
    nc = bacc.Bacc(
        "TRN2", target_bir_lowering=False, debug=False, num_devices=8
    )
    xT = nc.dram_tensor("xT", [D, S], BF16, kind="ExternalInput").ap()
    wq = nc.dram_tensor("wq", [D, DG], BF16, kind="ExternalInput").ap()
    wk = nc.dram_tensor("wk", [D, DG], BF16, kind="ExternalInput").ap()
    wv = nc.dram_tensor("wv", [D, DG], BF16, kind="ExternalInput").ap()
    wo = nc.dram_tensor("wo", [HG, HD, D], BF16, kind="ExternalInput").ap()
    bq = nc.dram_tensor("bq", [128, 2], F32, kind="ExternalInput").ap()
    out = nc.dram_tensor("out", [S, D], F32, kind="ExternalOutput").ap()
    with tile.TileContext(nc) as tc:
        _emit(tc, xT, wq, wk, wv, wo, bq, out)
    nc.compile()
    return nc


def _get_program():
    global _PROGRAM
    if _PROGRAM is None:
        _PROGRAM = _build_program()
    return _PROGRAM


def kernel(x, Wq, bq, Wk, bk, Wv, bv, Wo, bo):
    global LAST_RESULTS
    x = np.asarray(x, dtype=np.float32)
    Wq = np.asarray(Wq, dtype=np.float32)
    bq = np.asarray(bq, dtype=np.float32)
    Wk = np.asarray(Wk, dtype=np.float32)
    Wv = np.asarray(Wv, dtype=np.float32)
    bv = np.asarray(bv, dtype=np.float32)
    Wo = np.asarray(Wo, dtype=np.float32)
    bo = np.asarray(bo, dtype=np.float32)

    scale = 1.0 / np.sqrt(np.float32(HD)).astype(np.float32)
    nc = _get_program()

    xT_b = [np.ascontiguousarray(x[b].T).astype(NPBF16) for b in range(2)]
    in_maps = []
    for c in range(8):
        b, g = c // 4, c % 4
        hs = slice(DG * g, DG * (g + 1))
        in_maps.append(
            {
                "xT": xT_b[b],
                "wq": (Wq[:, hs] * scale).astype(NPBF16),
                "wk": np.ascontiguousarray(Wk[:, hs]).astype(NPBF16),
                "wv": np.ascontiguousarray(Wv[:, hs]).astype(NPBF16),
                "wo": np.ascontiguousarray(
                    Wo[hs, :].reshape(HG, HD, D)
                ).astype(NPBF16),
                "bq": np.ascontiguousarray(
                    (bq[hs] * scale).reshape(2, 128).T
                ).astype(np.float32),
            }
        )

    res = run_bass_kernel_spmd(
        nc, in_maps, list(range(8)), trace=TRACE, **TRACE_KWARGS
    )
    LAST_RESULTS = res
    parts = [r["out"] for r in res.results]
    const = (bv @ Wo + bo).astype(np.float32)
    y = np.empty((2, S, D), dtype=np.float32)
    for b in range(2):
        y[b] = parts[4 * b] + parts[4 * b + 1] + parts[4 * b + 2] + parts[4 * b + 3]
        y[b] += const
    return y


# revision 14
# speedup vs baseline: 2.2563x; 1.8947x over previous
"""Multi-head attention (B=2, S=2048, D=1024, H=16, Hd=64) on 8 trn2 cores.

Sharding: core c handles batch b = c // 4 and head group g = c % 4 (heads
4g..4g+3).  Each core computes a partial output  O_g @ Wo[256g:256g+256, :]
for its batch; the host sums the 4 partials per batch and adds the
constant vector  bv @ Wo + bo  (exact: softmax weights sum to 1, so the
V-bias contributes bv @ Wo; bk cancels inside softmax rows).

Device program (identical on all cores, SPMD over different data):
  - inputs (per core): xT [1024,2048] bf16 (x[b].T), wq/wk/wv [1024,256]
    bf16 (head-group column slices; wq and bq pre-scaled by 1/8),
    wo [4,64,1024] bf16 (head-major row slice of Wo), bq [128,2] f32.
  - Q^T,K^T = W^T @ x^T  (contraction over d_in on partitions),
    V = x @ Wv (natural orientation), with a constant ones column
    appended per head for softmax row sums.
  - per (head pair 2*j2..2*j2+1, 512-wide query chunk): S^T[k,q] via
    c=64 matmuls packed on PE row groups 0-63/64-127, exp on ScalarE
    (PSUM->SBUF bf16, no max subtraction: |scores| < ~3), O^T/sums
    accumulated over 16 k-blocks via lhsT=[V_h|1].
  - softmax denominators: sums row spread over 128 partitions via a
    small SBUF->SBUF DMA, reciprocal there (cheap), DMA back, then a
    c=1 PE matmul broadcasts 1/sum over partitions; one DVE multiply
    normalizes.  The broadcast matmul and the final projection are
    emitted one group late so the in-order PE queue never stalls on
    the normalization tail.
  - final: out[s,:] = sum_h O_h @ Wo_h  (c=64 accumulating matmuls).
"""

import numpy as np
import ml_dtypes

import concourse.bass as bass
import concourse.mybir as mybir
import concourse.tile as tile
from concourse import bacc
from concourse.bass_utils import run_bass_kernel_spmd

BF16 = mybir.dt.bfloat16
F32 = mybir.dt.float32
NPBF16 = ml_dtypes.bfloat16

S = 2048
D = 1024
HG = 4        # heads per core
HD = 64       # head dim
DG = HG * HD  # 256 projection columns per core

# Set by test.py to capture a profile; results stashed on LAST_RESULTS.
TRACE = False
TRACE_KWARGS = {}
LAST_RESULTS = None

_PROGRAM = None


def _emit(tc, xT, wq, wk, wv, wo, bq, out):
    nc = tc.nc
    from contextlib import ExitStack

    with ExitStack() as ctx:
        const = ctx.enter_context(tc.tile_pool(name="const", bufs=1))
        sb = ctx.enter_context(tc.tile_pool(name="sb", bufs=1))
        pts = ctx.enter_context(tc.tile_pool(name="pt", bufs=2))
        outp = ctx.enter_context(tc.tile_pool(name="outp", bufs=3))
        obp = ctx.enter_context(tc.tile_pool(name="ob", bufs=2))
        rsp = ctx.enter_context(tc.tile_pool(name="rs", bufs=2))
        rrp = ctx.enter_context(tc.tile_pool(name="rr", bufs=2))
        # 3 x 2-bank rotating PSUM tiles (all phases) + 2 PV banks = 8 banks
        psp = ctx.enter_context(
            tc.tile_pool(name="ps", bufs=3, space=bass.MemorySpace.PSUM)
        )
        pop = ctx.enter_context(
            tc.tile_pool(name="po", bufs=1, space=bass.MemorySpace.PSUM)
        )

        def pstile():
            return psp.tile([128, 2, 512], F32, name="ps", tag="ps")

        # ---- persistent SBUF tensors ----
        x_sb = sb.tile([128, 8, S], BF16)          # x^T, d_in blocks on partitions
        wq_sb = const.tile([128, 8, DG], BF16)
        wk_sb = const.tile([128, 8, DG], BF16)
        wv_sb = const.tile([128, 8, DG], BF16)
        wo_sb = const.tile([64, HG, D], BF16)      # head-major Wo rows
        bq_sb = const.tile([128, 2], F32)
        ones_sb = const.tile([128, 64], BF16)      # for 1/sum broadcast lhsT
        scr_sb = const.tile([1, 1], F32)
        qt_sb = sb.tile([128, 2, S], BF16)         # Q^T (d_out on partitions)
        kt_sb = sb.tile([128, 2, S], BF16)         # K^T
        v_sb = sb.tile([128, 16, HG, HD + 1], BF16)  # V by s-block, ones col last
        o_sb = sb.tile([64, HG, S], BF16)          # O^T per head (partitions 0-63)

        # ---- input DMAs (ordered so phase-1 compute can start early) ----
        nc.vector.memset(ones_sb[:], 1.0)
        nc.vector.memset(v_sb[:, :, :, HD : HD + 1], 1.0)
        # preload the exp table set while DMAs run
        nc.scalar.activation(
            out=scr_sb[:], in_=ones_sb[0:1, 0:1],
            func=mybir.ActivationFunctionType.Exp,
        )
        xT_r = xT.rearrange("(j p) s -> p j s", p=128)
        nc.sync.dma_start(out=wq_sb[:], in_=wq.rearrange("(j p) m -> p j m", p=128))
        nc.sync.dma_start(out=wk_sb[:], in_=wk.rearrange("(j p) m -> p j m", p=128))
        nc.sync.dma_start(out=bq_sb[:], in_=bq)
        nc.sync.dma_start(out=x_sb[:, :, 0:512], in_=xT_r[:, :, 0:512])
        nc.sync.dma_start(out=wv_sb[:], in_=wv.rearrange("(j p) m -> p j m", p=128))
        nc.sync.dma_start(out=wo_sb[:], in_=wo.rearrange("h p m -> p h m"))
        for c in range(1, 4):
            cs = slice(512 * c, 512 * (c + 1))
            nc.sync.dma_start(out=x_sb[:, :, cs], in_=xT_r[:, :, cs])

        # ---- phase 1: projections ----
        for c in range(4):
            cs = slice(512 * c, 512 * (c + 1))
            tq = pstile()
            for j2 in range(2):
                ms = slice(128 * j2, 128 * (j2 + 1))
                for j in range(8):
                    nc.tensor.matmul(
                        tq[:, j2, :], wq_sb[:, j, ms], x_sb[:, j, cs],
                        start=(j == 0), stop=(j == 7),
                    )
            for j2 in range(2):
                nc.vector.tensor_scalar_add(
                    qt_sb[:, j2, cs], tq[:, j2, :], bq_sb[:, j2 : j2 + 1]
                )
            tk = pstile()
            for j2 in range(2):
                ms = slice(128 * j2, 128 * (j2 + 1))
                for j in range(8):
                    nc.tensor.matmul(
                        tk[:, j2, :], wk_sb[:, j, ms], x_sb[:, j, cs],
                        start=(j == 0), stop=(j == 7),
                    )
            nc.vector.tensor_copy(kt_sb[:, :, cs], tk[:])
            for s2 in range(2):
                tv = pstile()
                for hh in range(2):
                    sblk = 4 * c + 2 * s2 + hh
                    ss = slice(128 * sblk, 128 * (sblk + 1))
                    for j in range(8):
                        nc.tensor.matmul(
                            tv[:, hh, 0:DG], x_sb[:, j, ss], wv_sb[:, j, :],
                            start=(j == 0), stop=(j == 7),
                        )
                for hh in range(2):
                    sblk = 4 * c + 2 * s2 + hh
                    nc.vector.tensor_copy(
                        v_sb[:, sblk, :, 0:HD],
                        tv[:, hh, 0:DG].rearrange("p (h d) -> p h d", h=HG),
                    )

        # ---- phase 2+3: attention, with norm-tail and final projection
        # deferred by one group to keep the PE queue unblocked ----

        def emit_final(c):
            for sblk in range(4 * c, 4 * c + 4):
                ss = slice(128 * sblk, 128 * (sblk + 1))
                ot = outp.tile([128, D], F32)
                ft = pstile()
                for half in range(2):
                    hs = slice(512 * half, 512 * (half + 1))
                    for h in range(HG):
                        nc.tensor.matmul(
                            ft[:, half, :], o_sb[:, h, ss], wo_sb[:, h, hs],
                            start=(h == 0), stop=(h == HG - 1),
                        )
                nc.vector.tensor_copy(ot[:], ft[:])
                nc.sync.dma_start(out=out[ss, :], in_=ot[:])

        pending = []  # deferred norm-tail emitters
        for c in range(4):
            cs = slice(512 * c, 512 * (c + 1))
            for j2 in range(2):
                pt = pts.tile([128, 16, 2, 512], BF16)
                po = pop.tile([128, 2, 512], F32)

                def pv(kblk, j2=j2, po=po, pt=pt):
                    for hh in range(2):
                        nc.tensor.matmul(
                            po[0 : HD + 1, hh, :],
                            v_sb[:, kblk, 2 * j2 + hh, :],
                            pt[:, kblk, hh, :],
                            start=(kblk == 0), stop=(kblk == 15),
                        )

                for kblk in range(16):
                    ks = slice(128 * kblk, 128 * (kblk + 1))
                    st = pstile()
                    for hh in range(2):
                        pp = slice(64 * hh, 64 * hh + 64)
                        nc.tensor.matmul(
                            st[:, hh, :],
                            kt_sb[pp, j2, ks],
                            qt_sb[pp, j2, cs],
                            start=True, stop=True,
                        )
                    nc.scalar.activation(
                        out=pt[:, kblk, :, :],
                        in_=st[:],
                        func=mybir.ActivationFunctionType.Exp,
                    )
                    if kblk >= 1:
                        pv(kblk - 1)
                pv(15)

                # evacuate PV banks; spread sums over partitions via DMA,
                # reciprocal there, cast to bf16, DMA back for the
                # (bf16, single-pass) broadcast matmul
                ob = obp.tile([HD + 1, 2, 512], F32)
                nc.vector.tensor_copy(ob[:], po[0 : HD + 1, :, :])
                rs = rsp.tile([128, 2, 4], F32)
                rc = rsp.tile([128, 2, 4], BF16)
                rr = rrp.tile([128, 2, 512], BF16)
                for hh in range(2):
                    nc.sync.dma_start(
                        out=rs[:, hh, :], in_=ob[HD : HD + 1, hh, :]
                    )
                nc.vector.reciprocal(rs[:], rs[:])
                nc.vector.tensor_copy(rc[:], rs[:])
                for hh in range(2):
                    nc.sync.dma_start(
                        out=rr[HD : HD + 1, hh, :], in_=rc[:, hh, :]
                    )

                def tail(c=c, cs=cs, j2=j2, ob=ob, rr=rr):
                    bt = pstile()
                    for hh in range(2):
                        nc.tensor.matmul(
                            bt[0:64, hh, :],
                            ones_sb[HD : HD + 1, :],
                            rr[HD : HD + 1, hh, :],
                            start=True, stop=True,
                        )
                    for hh in range(2):
                        nc.vector.tensor_mul(
                            o_sb[:, 2 * j2 + hh, cs],
                            ob[0:HD, hh, :],
                            bt[0:64, hh, :],
                        )

                pending.append((tail, c if j2 == 1 else None))
                if len(pending) >= 2:
                    t, fc = pending.pop(0)
                    t()
                    if fc is not None:
                        emit_final(fc)
        for t, fc in pending:
            t()
            if fc is not None:
                emit_final(fc)


def _build_program():
    nc = bacc.Bacc(
        "TRN2", target_bir_lowering=False, debug=False, num_devices=8
    )
    xT = nc.dram_tensor("xT", [D, S], BF16, kind="ExternalInput").ap()
    wq = nc.dram_tensor("wq", [D, DG], BF16, kind="ExternalInput").ap()
    wk = nc.dram_tensor("wk", [D, DG], BF16, kind="ExternalInput").ap()
    wv = nc.dram_tensor("wv", [D, DG], BF16, kind="ExternalInput").ap()
    wo = nc.dram_tensor("wo", [HG, HD, D], BF16, kind="ExternalInput").ap()
    bq = nc.dram_tensor("bq", [128, 2], F32, kind="ExternalInput").ap()
    out = nc.dram_tensor("out", [S, D], F32, kind="ExternalOutput").ap()
    with tile.TileContext(nc) as tc:
        _emit(tc, xT, wq, wk, wv, wo, bq, out)
    nc.compile()
    return nc


def _get_program():
    global _PROGRAM
    if _PROGRAM is None:
        _PROGRAM = _build_program()
    return _PROGRAM


def kernel(x, Wq, bq, Wk, bk, Wv, bv, Wo, bo):
    global LAST_RESULTS
    x = np.asarray(x, dtype=np.float32)
    Wq = np.asarray(Wq, dtype=np.float32)
    bq = np.asarray(bq, dtype=np.float32)
    Wk = np.asarray(Wk, dtype=np.float32)
    Wv = np.asarray(Wv, dtype=np.float32)
    bv = np.asarray(bv, dtype=np.float32)
    Wo = np.asarray(Wo, dtype=np.float32)
    bo = np.asarray(bo, dtype=np.float32)

    scale = 1.0 / np.sqrt(np.float32(HD)).astype(np.float32)
    nc = _get_program()

    xT_b = [np.ascontiguousarray(x[b].T).astype(NPBF16) for b in range(2)]
    in_maps = []
    for c in range(8):
        b, g = c // 4, c % 4
        hs = slice(DG * g, DG * (g + 1))
        in_maps.append(
            {
                "xT": xT_b[b],
                "wq": (Wq[:, hs] * scale).astype(NPBF16),
                "wk": np.ascontiguousarray(Wk[:, hs]).astype(NPBF16),
                "wv": np.ascontiguousarray(Wv[:, hs]).astype(NPBF16),
                "wo": np.ascontiguousarray(
                    Wo[hs, :].reshape(HG, HD, D)
                ).astype(NPBF16),
                "bq": np.ascontiguousarray(
                    (bq[hs] * scale).reshape(2, 128).T
                ).astype(np.float32),
            }
        )

    res = run_bass_kernel_spmd(
        nc, in_maps, list(range(8)), trace=TRACE, **TRACE_KWARGS
    )
    LAST_RESULTS = res
    parts = [r["out"] for r in res.results]
    const = (bv @ Wo + bo).astype(np.float32)
    y = np.empty((2, S, D), dtype=np.float32)
    for b in range(2):
        y[b] = parts[4 * b] + parts[4 * b + 1] + parts[4 * b + 2] + parts[4 * b + 3]
        y[b] += const
    return y
